# revision 21
# baseline (speedup 1.0000x reference)
"""MoE (8 experts, top-2, SwiGLU + shared expert) Trainium2 kernel.

Strategy: data-parallel over tokens across 8 NeuronCores (1024 tokens/core),
dense expert compute with the sparse combine weights folded into the
down-projection evacuation (scalar_tensor_tensor: acc += psum * c[t]).
All matmuls run as float32r (1 cycle/row at moving-dim 512). Weights are
streamed once per core (~63 MB). No collectives.

Layouts (per core):
  xT   [128(h_in), 8(h_out), 1024(t)]   x transposed via PE
  h    [128(d_in), 2(d_out), 1024(t)]   silu(x@Wg)*(x@Wu) for one 256-wide slice
  acc  [128(t_in), 8(t_out), 1024(h)]   final [t, h] accumulator
Sub-expert = 256 output channels: routed experts contribute 2 each,
the 1024-wide shared expert contributes 4 (combine weight 1.0).
"""

import numpy as np

E, K, H, D, NSH = 8, 2, 1024, 512, 2
DS = D * NSH
B, S = 4, 2048
T = B * S
NCORES = 8
TC = T // NCORES  # 1024 tokens per core
P = 128
DSUB = 256  # sub-expert width
HO = H // P  # 8
TO = TC // P  # 8
DC = DSUB // P  # 2 d-chunks per sub-expert
NT = 512  # moving-dim chunk (max for fp32)
TN = TC // NT  # 2
HH = H // NT  # 2

_CACHE = {}
SILU_MODE = "silu"  # "sigmoid" decomposes for CoreSim (no Silu there)


def _build():
    import concourse.bass as bass
    import concourse.mybir as mybir
    from concourse import bacc
    import concourse.tile as tile

    fp32 = mybir.dt.float32
    f32r = mybir.dt.float32r
    AF = mybir.ActivationFunctionType
    ALU = mybir.AluOpType
    AX = mybir.AxisListType

    def r(ap):  # tiles feeding matmuls are already float32r
        return ap

    nc = bacc.Bacc()
    xt = nc.declare_dram_parameter("xt", [H, TC], fp32, isOutput=False)
    wr = nc.declare_dram_parameter("wr", [H, E], fp32, isOutput=False)
    wg = nc.declare_dram_parameter("wg", [E, H, D], fp32, isOutput=False)
    wu = nc.declare_dram_parameter("wu", [E, H, D], fp32, isOutput=False)
    wd = nc.declare_dram_parameter("wd", [E, D, H], fp32, isOutput=False)
    wsg = nc.declare_dram_parameter("wsg", [H, DS], fp32, isOutput=False)
    wsu = nc.declare_dram_parameter("wsu", [H, DS], fp32, isOutput=False)
    wsd = nc.declare_dram_parameter("wsd", [DS, H], fp32, isOutput=False)
    out = nc.declare_dram_parameter("out", [TC, H], fp32, isOutput=True)
    auxp = nc.declare_dram_parameter("auxp", [1, E], fp32, isOutput=True)

    with tile.TileContext(nc) as tc:
        with (
            tc.tile_pool(name="const", bufs=1) as constp,
            tc.tile_pool(name="pers", bufs=1) as pers,
        ):
            ones_col = constp.tile([P, 1], fp32, tag="ones")
            nc.vector.memset(ones_col, 1.0)

            xTf = pers.tile([P, HO, TC], fp32, tag="xTf")
            xT = pers.tile([P, HO, TC], f32r, tag="xT")
            xt_r = xt[:, :].rearrange("(ho p) t -> p ho t", p=P)
            acc = pers.tile([P, TO, H], fp32, tag="acc")
            probs = pers.tile([P, TO, E], fp32, tag="probs")
            comb = pers.tile([P, TO, E], fp32, tag="comb")
            wr_sb = pers.tile([P, HO, E], fp32, tag="wr")
            nc.sync.dma_start(wr_sb, wr[:, :].rearrange("(ho p) e -> p ho e", p=P))

            # ---------- phase 1: transpose x, router, softmax, top-2 ----------
            def router_block(dnps, smallp):
                for to in range(TO):
                    rps_t = dnps.tile([P, NT], fp32, tag="d_ps")
                    rps = rps_t[:, :E]
                    for ho in range(HO):
                        nc.tensor.matmul(
                            rps,
                            xTf[:, ho, to * P:(to + 1) * P],
                            wr_sb[:, ho, :],
                            start=(ho == 0),
                            stop=(ho == HO - 1),
                        )
                    lmax = smallp.tile([P, 1], fp32, tag="lmax")
                    nc.vector.reduce_max(lmax, rps, AX.X)
                    nlmax = smallp.tile([P, 1], fp32, tag="nlmax")
                    nc.vector.tensor_scalar_mul(nlmax, lmax, -1.0)
                    pex = smallp.tile([P, E], fp32, tag="pex")
                    nc.scalar.activation(pex, rps, AF.Exp, bias=nlmax, scale=1.0)
                    ssum = smallp.tile([P, 1], fp32, tag="ssum")
                    nc.vector.reduce_sum(ssum, pex, AX.X)
                    rs = smallp.tile([P, 1], fp32, tag="rs")
                    nc.vector.reciprocal(rs, ssum)
                    nc.vector.tensor_scalar_mul(probs[:, to, :], pex, rs)
                    m1 = smallp.tile([P, 1], fp32, tag="m1")
                    nc.vector.reduce_max(m1, probs[:, to, :], AX.X)
                    # p2 = probs where not-top1 (probs - probs*mask1)
                    mask1 = smallp.tile([P, E], fp32, tag="mask1")
                    nc.vector.tensor_scalar(mask1, probs[:, to, :], m1, None, ALU.is_ge)
                    p2 = smallp.tile([P, E], fp32, tag="p2")
                    nc.vector.tensor_tensor(p2, probs[:, to, :], mask1, ALU.mult)
                    nc.vector.tensor_tensor(p2, probs[:, to, :], p2, ALU.subtract)
                    m2 = smallp.tile([P, 1], fp32, tag="m2")
                    nc.vector.reduce_max(m2, p2, AX.X)
                    den = smallp.tile([P, 1], fp32, tag="den")
                    nc.vector.tensor_tensor(den, m1, m2, ALU.add)
                    rden = smallp.tile([P, 1], fp32, tag="rden")
                    nc.vector.reciprocal(rden, den)
                    maskt = smallp.tile([P, E], fp32, tag="maskt")
                    nc.vector.tensor_scalar(maskt, probs[:, to, :], m2, None, ALU.is_ge)
                    cw = smallp.tile([P, E], fp32, tag="cw")
                    nc.vector.tensor_tensor(cw, probs[:, to, :], maskt, ALU.mult)
                    nc.vector.tensor_scalar_mul(comb[:, to, :], cw, rden)

            # ---------- phase 2: sub-experts ----------
            # subs: (kind, wg_ap, wu_ap, wd_ap, expert_idx or None)
            subs = []
            for sh in range(DS // DSUB):  # shared expert first (initializes acc)
                subs.append((
                    wsg[:, sh * DSUB:(sh + 1) * DSUB],
                    wsu[:, sh * DSUB:(sh + 1) * DSUB],
                    wsd[sh * DSUB:(sh + 1) * DSUB, :],
                    None,
                ))
            for e in range(E):
                for dh in range(D // DSUB):
                    subs.append((
                        wg[e, :, dh * DSUB:(dh + 1) * DSUB],
                        wu[e, :, dh * DSUB:(dh + 1) * DSUB],
                        wd[e, dh * DSUB:(dh + 1) * DSUB, :],
                        e,
                    ))

            with (
                tc.tile_pool(name="wguP", bufs=2) as wgup,
                tc.tile_pool(name="pre0", bufs=1) as pre0p,
                tc.tile_pool(name="wdP", bufs=2) as wdp,
                tc.tile_pool(name="hP", bufs=2) as hp,
                tc.tile_pool(name="gsP", bufs=4) as gsp,
                tc.tile_pool(name="guPS", bufs=2, space="PSUM") as gups,
                tc.tile_pool(name="dnPS", bufs=4, space="PSUM") as dnps,
                tc.tile_pool(name="small", bufs=3) as smallp,
            ):
                # sub-0 weights stream interleaved with x, per ho, so the DMA
                # queue (single FIFO) delivers the first gate fill's deps early
                wgt0 = pre0p.tile([P, HO, DSUB], f32r, tag="wg0")
                wut0 = pre0p.tile([P, HO, DSUB], f32r, tag="wu0")
                wg0r = subs[0][0].rearrange("(ho p) d -> p ho d", p=P).bitcast(f32r)
                wu0r = subs[0][1].rearrange("(ho p) d -> p ho d", p=P).bitcast(f32r)
                for ho in range(HO):
                    nc.sync.dma_start(xTf[:, ho, :], xt_r[:, ho, :])
                    nc.vector.tensor_copy(xT[:, ho, :], xTf[:, ho, :])
                    nc.sync.dma_start(wgt0[:, ho, :], wg0r[:, ho, :])
                    nc.sync.dma_start(wut0[:, ho, :], wu0r[:, ho, :])

                def do_sub(si, wg_ap, wu_ap, wd_ap, e):
                    if si == 0:
                        wgt, wut = wgt0, wut0
                    else:
                        wgt = wgup.tile([P, HO, DSUB], f32r, tag="wg_sb")
                        nc.sync.dma_start(wgt, wg_ap.rearrange("(ho p) d -> p ho d", p=P).bitcast(f32r))
                        wut = wgup.tile([P, HO, DSUB], f32r, tag="wu_sb")
                        nc.sync.dma_start(wut, wu_ap.rearrange("(ho p) d -> p ho d", p=P).bitcast(f32r))
                    wdt = wdp.tile([P, DC, H], f32r, tag="wd_sb")
                    nc.sync.dma_start(wdt, wd_ap.rearrange("(dc p) h -> p dc h", p=P).bitcast(f32r))

                    h_sb = hp.tile([P, DC, TC], f32r, tag="h_sb")
                    for dc in range(DC):
                        for tn in range(TN):
                            tsl = slice(tn * NT, (tn + 1) * NT)
                            gps = gups.tile([P, NT], fp32, tag="g_ps")
                            ups = gups.tile([P, NT], fp32, tag="u_ps")
                            for ho in range(HO):
                                nc.tensor.matmul(
                                    gps,
                                    r(wgt[:, ho, dc * P:(dc + 1) * P]),
                                    r(xT[:, ho, tsl]),
                                    start=(ho == 0), stop=(ho == HO - 1),
                                )
                            for ho in range(HO):
                                nc.tensor.matmul(
                                    ups,
                                    r(wut[:, ho, dc * P:(dc + 1) * P]),
                                    r(xT[:, ho, tsl]),
                                    start=(ho == 0), stop=(ho == HO - 1),
                                )
                            gs = gsp.tile([P, NT], fp32, tag="gs")
                            if SILU_MODE == "silu":
                                nc.scalar.activation(gs, gps, AF.Silu)
                            else:
                                nc.scalar.activation(gs, gps, AF.Sigmoid)
                                nc.vector.tensor_tensor(gs, gs, gps, ALU.mult)
                            nc.vector.tensor_tensor(h_sb[:, dc, tsl], gs, ups, ALU.mult)

                    for to in range(TO):
                        for hh in range(HH):
                            hsl = slice(hh * NT, (hh + 1) * NT)
                            dps = dnps.tile([P, NT], fp32, tag="d_ps")
                            for dc in range(DC):
                                nc.tensor.matmul(
                                    dps,
                                    r(h_sb[:, dc, to * P:(to + 1) * P]),
                                    r(wdt[:, dc, hsl]),
                                    start=(dc == 0), stop=(dc == DC - 1),
                                )
                            acc_sl = acc[:, to, hsl]
                            if si == 0:
                                # ACT drains PSUM straight into acc
                                nc.scalar.activation(acc_sl, dps, AF.Copy)
                            else:
                                # split evac: ACT reads PSUM (+combine scale),
                                # DVE adds SBUF->SBUF at 2x rate
                                tmp = gsp.tile([P, NT], fp32, tag="ev")
                                if e is None:
                                    nc.scalar.activation(tmp, dps, AF.Copy)
                                else:
                                    nc.scalar.activation(
                                        tmp, dps, AF.Copy,
                                        scale=comb[:, to, e:e + 1])
                                nc.vector.tensor_tensor(acc_sl, acc_sl, tmp, ALU.add)

                # shared sub 0 needs no router output: emit it first so PE
                # starts real work while the router/softmax chain runs
                do_sub(0, *subs[0])
                router_block(dnps, smallp)
                for si, (wg_ap, wu_ap, wd_ap, e) in enumerate(subs):
                    if si == 0:
                        continue
                    do_sub(si, wg_ap, wu_ap, wd_ap, e)

                for to in range(TO):
                    nc.sync.dma_start(out[to * P:(to + 1) * P, :], acc[:, to, :])

                # aux-loss partial at the tail (overlaps the drain/out DMA)
                aps_t = dnps.tile([P, NT], fp32, tag="d_ps")
                aps_ = aps_t[:1, :E]
                for to in range(TO):
                    nc.tensor.matmul(
                        aps_, ones_col, probs[:, to, :],
                        start=(to == 0), stop=(to == TO - 1),
                    )
                aux_sb = smallp.tile([1, E], fp32, tag="aux_sb")
                nc.vector.tensor_copy(aux_sb, aps_)
                nc.sync.dma_start(auxp[:, :], aux_sb)

    nc.compile()
    return nc


def _get_nc():
    if "nc" not in _CACHE:
        _CACHE["nc"] = _build()
    return _CACHE["nc"]


def _in_maps(inputs):
    f = lambda a: np.ascontiguousarray(np.asarray(a), dtype=np.float32)
    xf = f(inputs["x"]).reshape(T, H)
    shared = {
        "wr": f(inputs["W_router"]),
        "wg": f(inputs["Wg"]),
        "wu": f(inputs["Wu"]),
        "wd": f(inputs["Wd"]),
        "wsg": f(inputs["Wsg"]),
        "wsu": f(inputs["Wsu"]),
        "wsd": f(inputs["Wsd"]),
    }
    return [
        {"xt": np.ascontiguousarray(xf[i * TC:(i + 1) * TC].T), **shared}
        for i in range(NCORES)
    ]


def _postprocess(results):
    outf = np.concatenate([np.asarray(r["out"]) for r in results], axis=0)
    psum = np.zeros(E, np.float64)
    for r_ in results:
        psum += np.asarray(r_["auxp"], np.float64).reshape(E)
    mean = (psum / T).astype(np.float32)
    aux = np.float32(E * np.sum(mean * mean))
    return outf.reshape(B, S, H), aux


def kernel(**inputs):
    from concourse.bass_utils import run_bass_kernel_spmd

    nc = _get_nc()
    res = run_bass_kernel_spmd(nc, _in_maps(inputs), list(range(NCORES)))
    return _postprocess(res.results)


# revision 22
# speedup vs baseline: 1.0247x; 1.0247x over previous
"""MoE (8 experts, top-2, SwiGLU + shared expert) Trainium2 kernel.

Strategy: data-parallel over tokens across 8 NeuronCores (1024 tokens/core),
dense expert compute with the sparse combine weights folded into the
down-projection evacuation (scalar_tensor_tensor: acc += psum * c[t]).
All matmuls run as float32r (1 cycle/row at moving-dim 512). Weights are
streamed once per core (~63 MB). No collectives.

Layouts (per core):
  xT   [128(h_in), 8(h_out), 1024(t)]   x transposed via PE
  h    [128(d_in), 2(d_out), 1024(t)]   silu(x@Wg)*(x@Wu) for one 256-wide slice
  acc  [128(t_in), 8(t_out), 1024(h)]   final [t, h] accumulator
Sub-expert = 256 output channels: routed experts contribute 2 each,
the 1024-wide shared expert contributes 4 (combine weight 1.0).
"""

import numpy as np

E, K, H, D, NSH = 8, 2, 1024, 512, 2
DS = D * NSH
B, S = 4, 2048
T = B * S
NCORES = 8
TC = T // NCORES  # 1024 tokens per core
P = 128
DSUB = 256  # sub-expert width
HO = H // P  # 8
TO = TC // P  # 8
DC = DSUB // P  # 2 d-chunks per sub-expert
NT = 512  # moving-dim chunk (max for fp32)
TN = TC // NT  # 2
HH = H // NT  # 2

_CACHE = {}
SILU_MODE = "silu"  # "sigmoid" decomposes for CoreSim (no Silu there)


def _build():
    import concourse.bass as bass
    import concourse.mybir as mybir
    from concourse import bacc
    import concourse.tile as tile

    fp32 = mybir.dt.float32
    f32r = mybir.dt.float32r
    AF = mybir.ActivationFunctionType
    ALU = mybir.AluOpType
    AX = mybir.AxisListType

    def r(ap):  # tiles feeding matmuls are already float32r
        return ap

    nc = bacc.Bacc()
    xt = nc.declare_dram_parameter("xt", [H, TC], fp32, isOutput=False)
    wr = nc.declare_dram_parameter("wr", [H, E], fp32, isOutput=False)
    wg = nc.declare_dram_parameter("wg", [E, H, D], fp32, isOutput=False)
    wu = nc.declare_dram_parameter("wu", [E, H, D], fp32, isOutput=False)
    wd = nc.declare_dram_parameter("wd", [E, D, H], fp32, isOutput=False)
    wsg = nc.declare_dram_parameter("wsg", [H, DS], fp32, isOutput=False)
    wsu = nc.declare_dram_parameter("wsu", [H, DS], fp32, isOutput=False)
    wsd = nc.declare_dram_parameter("wsd", [DS, H], fp32, isOutput=False)
    out = nc.declare_dram_parameter("out", [TC, H], fp32, isOutput=True)
    auxp = nc.declare_dram_parameter("auxp", [1, E], fp32, isOutput=True)

    with tile.TileContext(nc) as tc:
        with (
            tc.tile_pool(name="const", bufs=1) as constp,
            tc.tile_pool(name="pers", bufs=1) as pers,
        ):
            ones_col = constp.tile([P, 1], fp32, tag="ones")
            nc.vector.memset(ones_col, 1.0)

            xTf = pers.tile([P, HO, TC], fp32, tag="xTf")
            xT = pers.tile([P, HO, TC], f32r, tag="xT")
            xt_r = xt[:, :].rearrange("(ho p) t -> p ho t", p=P)
            acc = pers.tile([P, TO, H], fp32, tag="acc")
            probs = pers.tile([P, TO, E], fp32, tag="probs")
            comb = pers.tile([P, TO, E], fp32, tag="comb")
            wr_sb = pers.tile([P, HO, E], fp32, tag="wr")
            nc.sync.dma_start(wr_sb, wr[:, :].rearrange("(ho p) e -> p ho e", p=P))

            # ---------- phase 1: transpose x, router, softmax, top-2 ----------
            def router_block(dnps, smallp):
                for to in range(TO):
                    rps_t = dnps.tile([P, NT], fp32, tag="d_ps")
                    rps = rps_t[:, :E]
                    for ho in range(HO):
                        nc.tensor.matmul(
                            rps,
                            xTf[:, ho, to * P:(to + 1) * P],
                            wr_sb[:, ho, :],
                            start=(ho == 0),
                            stop=(ho == HO - 1),
                        )
                    lmax = smallp.tile([P, 1], fp32, tag="lmax")
                    nc.vector.reduce_max(lmax, rps, AX.X)
                    nlmax = smallp.tile([P, 1], fp32, tag="nlmax")
                    nc.vector.tensor_scalar_mul(nlmax, lmax, -1.0)
                    pex = smallp.tile([P, E], fp32, tag="pex")
                    nc.scalar.activation(pex, rps, AF.Exp, bias=nlmax, scale=1.0)
                    ssum = smallp.tile([P, 1], fp32, tag="ssum")
                    nc.vector.reduce_sum(ssum, pex, AX.X)
                    rs = smallp.tile([P, 1], fp32, tag="rs")
                    nc.vector.reciprocal(rs, ssum)
                    nc.vector.tensor_scalar_mul(probs[:, to, :], pex, rs)
                    m1 = smallp.tile([P, 1], fp32, tag="m1")
                    nc.vector.reduce_max(m1, probs[:, to, :], AX.X)
                    # p2 = probs where not-top1 (probs - probs*mask1)
                    mask1 = smallp.tile([P, E], fp32, tag="mask1")
                    nc.vector.tensor_scalar(mask1, probs[:, to, :], m1, None, ALU.is_ge)
                    p2 = smallp.tile([P, E], fp32, tag="p2")
                    nc.vector.tensor_tensor(p2, probs[:, to, :], mask1, ALU.mult)
                    nc.vector.tensor_tensor(p2, probs[:, to, :], p2, ALU.subtract)
                    m2 = smallp.tile([P, 1], fp32, tag="m2")
                    nc.vector.reduce_max(m2, p2, AX.X)
                    den = smallp.tile([P, 1], fp32, tag="den")
                    nc.vector.tensor_tensor(den, m1, m2, ALU.add)
                    rden = smallp.tile([P, 1], fp32, tag="rden")
                    nc.vector.reciprocal(rden, den)
                    maskt = smallp.tile([P, E], fp32, tag="maskt")
                    nc.vector.tensor_scalar(maskt, probs[:, to, :], m2, None, ALU.is_ge)
                    cw = smallp.tile([P, E], fp32, tag="cw")
                    nc.vector.tensor_tensor(cw, probs[:, to, :], maskt, ALU.mult)
                    nc.vector.tensor_scalar_mul(comb[:, to, :], cw, rden)

            # ---------- phase 2: sub-experts ----------
            # subs: (kind, wg_ap, wu_ap, wd_ap, expert_idx or None)
            subs = []
            for sh in range(DS // DSUB):  # shared expert first (initializes acc)
                subs.append((
                    wsg[:, sh * DSUB:(sh + 1) * DSUB],
                    wsu[:, sh * DSUB:(sh + 1) * DSUB],
                    wsd[sh * DSUB:(sh + 1) * DSUB, :],
                    None,
                ))
            for e in range(E):
                for dh in range(D // DSUB):
                    subs.append((
                        wg[e, :, dh * DSUB:(dh + 1) * DSUB],
                        wu[e, :, dh * DSUB:(dh + 1) * DSUB],
                        wd[e, dh * DSUB:(dh + 1) * DSUB, :],
                        e,
                    ))

            with (
                tc.tile_pool(name="wguP", bufs=2) as wgup,
                tc.tile_pool(name="pre0", bufs=1) as pre0p,
                tc.tile_pool(name="wdP", bufs=2) as wdp,
                tc.tile_pool(name="hP", bufs=2) as hp,
                tc.tile_pool(name="gsP", bufs=4) as gsp,
                tc.tile_pool(name="guPS", bufs=2, space="PSUM") as gups,
                tc.tile_pool(name="dnPS", bufs=4, space="PSUM") as dnps,
                tc.tile_pool(name="small", bufs=3) as smallp,
            ):
                # sub-0 weights stream interleaved with x, per ho, so the DMA
                # queue (single FIFO) delivers the first gate fill's deps early
                wgt0 = pre0p.tile([P, HO, DSUB], f32r, tag="wg0")
                wut0 = pre0p.tile([P, HO, DSUB], f32r, tag="wu0")
                wg0r = subs[0][0].rearrange("(ho p) d -> p ho d", p=P).bitcast(f32r)
                wu0r = subs[0][1].rearrange("(ho p) d -> p ho d", p=P).bitcast(f32r)
                for ho in range(HO):
                    nc.sync.dma_start(xTf[:, ho, :], xt_r[:, ho, :])
                    nc.vector.tensor_copy(xT[:, ho, :], xTf[:, ho, :])
                    nc.sync.dma_start(wgt0[:, ho, :], wg0r[:, ho, :])
                    nc.sync.dma_start(wut0[:, ho, :], wu0r[:, ho, :])

                def do_sub(si, wg_ap, wu_ap, wd_ap, e):
                    if si == 0:
                        wgt, wut = wgt0, wut0
                    else:
                        wgt = wgup.tile([P, HO, DSUB], f32r, tag="wg_sb")
                        nc.sync.dma_start(wgt, wg_ap.rearrange("(ho p) d -> p ho d", p=P).bitcast(f32r))
                        wut = wgup.tile([P, HO, DSUB], f32r, tag="wu_sb")
                        nc.sync.dma_start(wut, wu_ap.rearrange("(ho p) d -> p ho d", p=P).bitcast(f32r))
                    wdt = wdp.tile([P, DC, H], f32r, tag="wd_sb")
                    nc.sync.dma_start(wdt, wd_ap.rearrange("(dc p) h -> p dc h", p=P).bitcast(f32r))

                    h_sb = hp.tile([P, DC, TC], f32r, tag="h_sb")
                    for dc in range(DC):
                        for tn in range(TN):
                            tsl = slice(tn * NT, (tn + 1) * NT)
                            gps = gups.tile([P, NT], fp32, tag="g_ps")
                            ups = gups.tile([P, NT], fp32, tag="u_ps")
                            for ho in range(HO):
                                nc.tensor.matmul(
                                    gps,
                                    r(wgt[:, ho, dc * P:(dc + 1) * P]),
                                    r(xT[:, ho, tsl]),
                                    start=(ho == 0), stop=(ho == HO - 1),
                                )
                            for ho in range(HO):
                                nc.tensor.matmul(
                                    ups,
                                    r(wut[:, ho, dc * P:(dc + 1) * P]),
                                    r(xT[:, ho, tsl]),
                                    start=(ho == 0), stop=(ho == HO - 1),
                                )
                            gs = gsp.tile([P, NT], fp32, tag="gs")
                            if SILU_MODE == "silu":
                                nc.scalar.activation(gs, gps, AF.Silu)
                            else:
                                nc.scalar.activation(gs, gps, AF.Sigmoid)
                                nc.vector.tensor_tensor(gs, gs, gps, ALU.mult)
                            nc.vector.tensor_tensor(h_sb[:, dc, tsl], gs, ups, ALU.mult)

                    for to in range(TO):
                        for hh in range(HH):
                            hsl = slice(hh * NT, (hh + 1) * NT)
                            dps = dnps.tile([P, NT], fp32, tag="d_ps")
                            for dc in range(DC):
                                nc.tensor.matmul(
                                    dps,
                                    r(h_sb[:, dc, to * P:(to + 1) * P]),
                                    r(wdt[:, dc, hsl]),
                                    start=(dc == 0), stop=(dc == DC - 1),
                                )
                            acc_sl = acc[:, to, hsl]
                            if si == 0:
                                nc.vector.tensor_copy(acc_sl, dps)
                            elif e is None:
                                nc.vector.tensor_tensor(acc_sl, acc_sl, dps, ALU.add)
                            else:
                                # acc += psum * combine[t, e]
                                nc.vector.scalar_tensor_tensor(
                                    acc_sl, dps, comb[:, to, e:e + 1], acc_sl,
                                    ALU.mult, ALU.add,
                                )

                # shared sub 0 needs no router output: emit it first so PE
                # starts real work while the router/softmax chain runs
                do_sub(0, *subs[0])
                router_block(dnps, smallp)
                for si, (wg_ap, wu_ap, wd_ap, e) in enumerate(subs):
                    if si == 0:
                        continue
                    do_sub(si, wg_ap, wu_ap, wd_ap, e)

                for to in range(TO):
                    nc.sync.dma_start(out[to * P:(to + 1) * P, :], acc[:, to, :])

                # aux-loss partial at the tail (overlaps the drain/out DMA)
                aps_t = dnps.tile([P, NT], fp32, tag="d_ps")
                aps_ = aps_t[:1, :E]
                for to in range(TO):
                    nc.tensor.matmul(
                        aps_, ones_col, probs[:, to, :],
                        start=(to == 0), stop=(to == TO - 1),
                    )
                aux_sb = smallp.tile([1, E], fp32, tag="aux_sb")
                nc.vector.tensor_copy(aux_sb, aps_)
                nc.sync.dma_start(auxp[:, :], aux_sb)

    nc.compile()
    return nc


def _get_nc():
    if "nc" not in _CACHE:
        _CACHE["nc"] = _build()
    return _CACHE["nc"]


def _in_maps(inputs):
    f = lambda a: np.ascontiguousarray(np.asarray(a), dtype=np.float32)
    xf = f(inputs["x"]).reshape(T, H)
    shared = {
        "wr": f(inputs["W_router"]),
        "wg": f(inputs["Wg"]),
        "wu": f(inputs["Wu"]),
        "wd": f(inputs["Wd"]),
        "wsg": f(inputs["Wsg"]),
        "wsu": f(inputs["Wsu"]),
        "wsd": f(inputs["Wsd"]),
    }
    return [
        {"xt": np.ascontiguousarray(xf[i * TC:(i + 1) * TC].T), **shared}
        for i in range(NCORES)
    ]


def _postprocess(results):
    outf = np.concatenate([np.asarray(r["out"]) for r in results], axis=0)
    psum = np.zeros(E, np.float64)
    for r_ in results:
        psum += np.asarray(r_["auxp"], np.float64).reshape(E)
    mean = (psum / T).astype(np.float32)
    aux = np.float32(E * np.sum(mean * mean))
    return outf.reshape(B, S, H), aux


def kernel(**inputs):
    from concourse.bass_utils import run_bass_kernel_spmd

    nc = _get_nc()
    res = run_bass_kernel_spmd(nc, _in_maps(inputs), list(range(NCORES)))
    return _postprocess(res.results)


# revision 23
# speedup vs baseline: 1.0322x; 1.0073x over previous
"""MoE (8 experts, top-2, SwiGLU + shared expert) Trainium2 kernel.

Strategy: data-parallel over tokens across 8 NeuronCores (1024 tokens/core),
dense expert compute with the sparse combine weights folded into the
down-projection evacuation (scalar_tensor_tensor: acc += psum * c[t]).
All matmuls run as float32r (1 cycle/row at moving-dim 512). Weights are
streamed once per core (~63 MB). No collectives.

Layouts (per core):
  xT   [128(h_in), 8(h_out), 1024(t)]   x transposed via PE
  h    [128(d_in), 2(d_out), 1024(t)]   silu(x@Wg)*(x@Wu) for one 256-wide slice
  acc  [128(t_in), 8(t_out), 1024(h)]   final [t, h] accumulator
Sub-expert = 256 output channels: routed experts contribute 2 each,
the 1024-wide shared expert contributes 4 (combine weight 1.0).
"""

import numpy as np

E, K, H, D, NSH = 8, 2, 1024, 512, 2
DS = D * NSH
B, S = 4, 2048
T = B * S
NCORES = 8
TC = T // NCORES  # 1024 tokens per core
P = 128
DSUB = 256  # sub-expert width
HO = H // P  # 8
TO = TC // P  # 8
DC = DSUB // P  # 2 d-chunks per sub-expert
NT = 512  # moving-dim chunk (max for fp32)
TN = TC // NT  # 2
HH = H // NT  # 2

_CACHE = {}
SILU_MODE = "silu"  # "sigmoid" decomposes for CoreSim (no Silu there)


def _build():
    import concourse.bass as bass
    import concourse.mybir as mybir
    from concourse import bacc
    import concourse.tile as tile

    fp32 = mybir.dt.float32
    f32r = mybir.dt.float32r
    AF = mybir.ActivationFunctionType
    ALU = mybir.AluOpType
    AX = mybir.AxisListType

    def r(ap):  # tiles feeding matmuls are already float32r
        return ap

    nc = bacc.Bacc()
    xt = nc.declare_dram_parameter("xt", [H, TC], fp32, isOutput=False)
    wr = nc.declare_dram_parameter("wr", [H, E], fp32, isOutput=False)
    wg = nc.declare_dram_parameter("wg", [E, H, D], fp32, isOutput=False)
    wu = nc.declare_dram_parameter("wu", [E, H, D], fp32, isOutput=False)
    wd = nc.declare_dram_parameter("wd", [E, D, H], fp32, isOutput=False)
    wsg = nc.declare_dram_parameter("wsg", [H, DS], fp32, isOutput=False)
    wsu = nc.declare_dram_parameter("wsu", [H, DS], fp32, isOutput=False)
    wsd = nc.declare_dram_parameter("wsd", [DS, H], fp32, isOutput=False)
    out = nc.declare_dram_parameter("out", [TC, H], fp32, isOutput=True)
    auxp = nc.declare_dram_parameter("auxp", [1, E], fp32, isOutput=True)

    with tile.TileContext(nc) as tc:
        with (
            tc.tile_pool(name="const", bufs=1) as constp,
            tc.tile_pool(name="pers", bufs=1) as pers,
        ):
            ones_col = constp.tile([P, 1], fp32, tag="ones")
            nc.vector.memset(ones_col, 1.0)

            xTf = pers.tile([P, HO, TC], fp32, tag="xTf")
            xT = pers.tile([P, HO, TC], f32r, tag="xT")
            xt_r = xt[:, :].rearrange("(ho p) t -> p ho t", p=P)
            acc = pers.tile([P, TO, H], fp32, tag="acc")
            probs = pers.tile([P, TO, E], fp32, tag="probs")
            comb = pers.tile([P, TO, E], fp32, tag="comb")
            wr_sb = pers.tile([P, HO, E], fp32, tag="wr")
            nc.sync.dma_start(wr_sb, wr[:, :].rearrange("(ho p) e -> p ho e", p=P))

            # ---------- phase 1: transpose x, router, softmax, top-2 ----------
            def router_block(dnps, smallp):
                for to in range(TO):
                    rps_t = dnps.tile([P, NT], fp32, tag="d_ps")
                    rps = rps_t[:, :E]
                    for ho in range(HO):
                        nc.tensor.matmul(
                            rps,
                            xTf[:, ho, to * P:(to + 1) * P],
                            wr_sb[:, ho, :],
                            start=(ho == 0),
                            stop=(ho == HO - 1),
                        )
                    lmax = smallp.tile([P, 1], fp32, tag="lmax")
                    nc.vector.reduce_max(lmax, rps, AX.X)
                    nlmax = smallp.tile([P, 1], fp32, tag="nlmax")
                    nc.vector.tensor_scalar_mul(nlmax, lmax, -1.0)
                    pex = smallp.tile([P, E], fp32, tag="pex")
                    nc.scalar.activation(pex, rps, AF.Exp, bias=nlmax, scale=1.0)
                    ssum = smallp.tile([P, 1], fp32, tag="ssum")
                    nc.vector.reduce_sum(ssum, pex, AX.X)
                    rs = smallp.tile([P, 1], fp32, tag="rs")
                    nc.vector.reciprocal(rs, ssum)
                    nc.vector.tensor_scalar_mul(probs[:, to, :], pex, rs)
                    m1 = smallp.tile([P, 1], fp32, tag="m1")
                    nc.vector.reduce_max(m1, probs[:, to, :], AX.X)
                    # p2 = probs where not-top1 (probs - probs*mask1)
                    mask1 = smallp.tile([P, E], fp32, tag="mask1")
                    nc.vector.tensor_scalar(mask1, probs[:, to, :], m1, None, ALU.is_ge)
                    p2 = smallp.tile([P, E], fp32, tag="p2")
                    nc.vector.tensor_tensor(p2, probs[:, to, :], mask1, ALU.mult)
                    nc.vector.tensor_tensor(p2, probs[:, to, :], p2, ALU.subtract)
                    m2 = smallp.tile([P, 1], fp32, tag="m2")
                    nc.vector.reduce_max(m2, p2, AX.X)
                    den = smallp.tile([P, 1], fp32, tag="den")
                    nc.vector.tensor_tensor(den, m1, m2, ALU.add)
                    rden = smallp.tile([P, 1], fp32, tag="rden")
                    nc.vector.reciprocal(rden, den)
                    maskt = smallp.tile([P, E], fp32, tag="maskt")
                    nc.vector.tensor_scalar(maskt, probs[:, to, :], m2, None, ALU.is_ge)
                    cw = smallp.tile([P, E], fp32, tag="cw")
                    nc.vector.tensor_tensor(cw, probs[:, to, :], maskt, ALU.mult)
                    nc.vector.tensor_scalar_mul(comb[:, to, :], cw, rden)

            # ---------- phase 2: sub-experts ----------
            # subs: (kind, wg_ap, wu_ap, wd_ap, expert_idx or None)
            subs = []
            for sh in range(DS // DSUB):  # shared expert first (initializes acc)
                subs.append((
                    wsg[:, sh * DSUB:(sh + 1) * DSUB],
                    wsu[:, sh * DSUB:(sh + 1) * DSUB],
                    wsd[sh * DSUB:(sh + 1) * DSUB, :],
                    None,
                ))
            for e in range(E):
                for dh in range(D // DSUB):
                    subs.append((
                        wg[e, :, dh * DSUB:(dh + 1) * DSUB],
                        wu[e, :, dh * DSUB:(dh + 1) * DSUB],
                        wd[e, dh * DSUB:(dh + 1) * DSUB, :],
                        e,
                    ))

            with (
                tc.tile_pool(name="wguP", bufs=3) as wgup,
                tc.tile_pool(name="pre0", bufs=1) as pre0p,
                tc.tile_pool(name="wdP", bufs=2) as wdp,
                tc.tile_pool(name="hP", bufs=2) as hp,
                tc.tile_pool(name="gsP", bufs=4) as gsp,
                tc.tile_pool(name="guPS", bufs=2, space="PSUM") as gups,
                tc.tile_pool(name="dnPS", bufs=4, space="PSUM") as dnps,
                tc.tile_pool(name="small", bufs=3) as smallp,
            ):
                # sub-0 weights stream interleaved with x, per ho, so the DMA
                # queue (single FIFO) delivers the first gate fill's deps early
                wgt0 = pre0p.tile([P, HO, DSUB], f32r, tag="wg0")
                wut0 = pre0p.tile([P, HO, DSUB], f32r, tag="wu0")
                wg0r = subs[0][0].rearrange("(ho p) d -> p ho d", p=P).bitcast(f32r)
                wu0r = subs[0][1].rearrange("(ho p) d -> p ho d", p=P).bitcast(f32r)
                for ho in range(HO):
                    nc.sync.dma_start(xTf[:, ho, :], xt_r[:, ho, :])
                    nc.vector.tensor_copy(xT[:, ho, :], xTf[:, ho, :])
                    nc.sync.dma_start(wgt0[:, ho, :], wg0r[:, ho, :])
                    nc.sync.dma_start(wut0[:, ho, :], wu0r[:, ho, :])

                def do_sub(si, wg_ap, wu_ap, wd_ap, e):
                    if si == 0:
                        wgt, wut = wgt0, wut0
                    else:
                        wgt = wgup.tile([P, HO, DSUB], f32r, tag="wg_sb")
                        nc.sync.dma_start(wgt, wg_ap.rearrange("(ho p) d -> p ho d", p=P).bitcast(f32r))
                        wut = wgup.tile([P, HO, DSUB], f32r, tag="wu_sb")
                        nc.sync.dma_start(wut, wu_ap.rearrange("(ho p) d -> p ho d", p=P).bitcast(f32r))
                    wdt = wdp.tile([P, DC, H], f32r, tag="wd_sb")
                    nc.sync.dma_start(wdt, wd_ap.rearrange("(dc p) h -> p dc h", p=P).bitcast(f32r))

                    h_sb = hp.tile([P, DC, TC], f32r, tag="h_sb")
                    for dc in range(DC):
                        for tn in range(TN):
                            tsl = slice(tn * NT, (tn + 1) * NT)
                            gps = gups.tile([P, NT], fp32, tag="g_ps")
                            ups = gups.tile([P, NT], fp32, tag="u_ps")
                            for ho in range(HO):
                                nc.tensor.matmul(
                                    gps,
                                    r(wgt[:, ho, dc * P:(dc + 1) * P]),
                                    r(xT[:, ho, tsl]),
                                    start=(ho == 0), stop=(ho == HO - 1),
                                )
                            for ho in range(HO):
                                nc.tensor.matmul(
                                    ups,
                                    r(wut[:, ho, dc * P:(dc + 1) * P]),
                                    r(xT[:, ho, tsl]),
                                    start=(ho == 0), stop=(ho == HO - 1),
                                )
                            gs = gsp.tile([P, NT], fp32, tag="gs")
                            if SILU_MODE == "silu":
                                nc.scalar.activation(gs, gps, AF.Silu)
                            else:
                                nc.scalar.activation(gs, gps, AF.Sigmoid)
                                nc.vector.tensor_tensor(gs, gs, gps, ALU.mult)
                            nc.vector.tensor_tensor(h_sb[:, dc, tsl], gs, ups, ALU.mult)

                    for to in range(TO):
                        for hh in range(HH):
                            hsl = slice(hh * NT, (hh + 1) * NT)
                            dps = dnps.tile([P, NT], fp32, tag="d_ps")
                            for dc in range(DC):
                                nc.tensor.matmul(
                                    dps,
                                    r(h_sb[:, dc, to * P:(to + 1) * P]),
                                    r(wdt[:, dc, hsl]),
                                    start=(dc == 0), stop=(dc == DC - 1),
                                )
                            acc_sl = acc[:, to, hsl]
                            if si == 0:
                                nc.vector.tensor_copy(acc_sl, dps)
                            elif e is None:
                                nc.vector.tensor_tensor(acc_sl, acc_sl, dps, ALU.add)
                            else:
                                # acc += psum * combine[t, e]
                                nc.vector.scalar_tensor_tensor(
                                    acc_sl, dps, comb[:, to, e:e + 1], acc_sl,
                                    ALU.mult, ALU.add,
                                )

                # shared sub 0 needs no router output: emit it first so PE
                # starts real work while the router/softmax chain runs
                do_sub(0, *subs[0])
                router_block(dnps, smallp)
                for si, (wg_ap, wu_ap, wd_ap, e) in enumerate(subs):
                    if si == 0:
                        continue
                    do_sub(si, wg_ap, wu_ap, wd_ap, e)

                for to in range(TO):
                    nc.sync.dma_start(out[to * P:(to + 1) * P, :], acc[:, to, :])

                # aux-loss partial at the tail (overlaps the drain/out DMA)
                aps_t = dnps.tile([P, NT], fp32, tag="d_ps")
                aps_ = aps_t[:1, :E]
                for to in range(TO):
                    nc.tensor.matmul(
                        aps_, ones_col, probs[:, to, :],
                        start=(to == 0), stop=(to == TO - 1),
                    )
                aux_sb = smallp.tile([1, E], fp32, tag="aux_sb")
                nc.vector.tensor_copy(aux_sb, aps_)
                nc.sync.dma_start(auxp[:, :], aux_sb)

    nc.compile()
    return nc


def _get_nc():
    if "nc" not in _CACHE:
        _CACHE["nc"] = _build()
    return _CACHE["nc"]


def _in_maps(inputs):
    f = lambda a: np.ascontiguousarray(np.asarray(a), dtype=np.float32)
    xf = f(inputs["x"]).reshape(T, H)
    shared = {
        "wr": f(inputs["W_router"]),
        "wg": f(inputs["Wg"]),
        "wu": f(inputs["Wu"]),
        "wd": f(inputs["Wd"]),
        "wsg": f(inputs["Wsg"]),
        "wsu": f(inputs["Wsu"]),
        "wsd": f(inputs["Wsd"]),
    }
    return [
        {"xt": np.ascontiguousarray(xf[i * TC:(i + 1) * TC].T), **shared}
        for i in range(NCORES)
    ]


def _postprocess(results):
    outf = np.concatenate([np.asarray(r["out"]) for r in results], axis=0)
    psum = np.zeros(E, np.float64)
    for r_ in results:
        psum += np.asarray(r_["auxp"], np.float64).reshape(E)
    mean = (psum / T).astype(np.float32)
    aux = np.float32(E * np.sum(mean * mean))
    return outf.reshape(B, S, H), aux


def kernel(**inputs):
    from concourse.bass_utils import run_bass_kernel_spmd

    nc = _get_nc()
    res = run_bass_kernel_spmd(nc, _in_maps(inputs), list(range(NCORES)))
    return _postprocess(res.results)
